# revision 1
# baseline (speedup 1.0000x reference)
"""Trainium2 Bass kernel for nn_DenseReparam.

Reference computation (fp32):
    angles = theta_lambda[:-2]            # [4095, 4096]
    lam    = theta_lambda[-2]             # [4096]
    r      = theta_lambda[-1]             # [4096]
    s, c   = sin(angles), cos(angles)
    cp     = cumprod(s, axis=0)
    v      = [c[0]; c[1:]*cp[:-1]; cp[-1]]   # [4096, 4096]
    z      = x @ v + lam                     # [8192, 4096]
    out    = r * relu(z)

Key numerical fact exploited here: cp decays like exp(-0.75*k) (angles are
standard normal), so in fp32 cp underflows to exactly 0 by row ~231 for every
column.  All v rows >= 232 are exact zeros and contribute nothing to x @ v.
We therefore truncate the contraction dim from 4096 to K_EFF = 320 (89 rows of
slack beyond the observed last nonzero row), which the kernel verifies is far
past fp32's denormal floor for these inputs.

Sharding (8 cores): batch split 2 x units split 4.  Each core computes
zT_local = r * relu(v_g^T @ x_b^T + lam) with shape [1024 units, 4096 batch]
(transposed layout so lam/r are per-partition scalars for the ACT/DVE
epilogue).  Host reassembles out[b, g] = zT_local^T.

Matmul runs in float32r (full fp32 precision, full PE rate at free dim >= 256).
cumprod is a DVE tensor_tensor_scan along the free dim in the units-transposed
layout; sin/cos come from the ACT Sin LUT (cos(x) = sin(x + pi/2)).
"""

import sys

import numpy as np

for _p in ("/root/.axon_site", "/root/.axon_site/_ro/trn_rl_repo",
           "/root/.axon_site/_ro/pypackages", "/opt/trn_rl_repo"):
    if _p not in sys.path:
        sys.path.append(_p)

from contextlib import ExitStack

from concourse import bass, mybir, tile
from concourse.bass_utils import run_bass_kernel_spmd
from concourse.masks import make_identity
from concourse.tile import add_dep_helper

F32 = mybir.dt.float32
F32R = mybir.dt.float32r
BF16 = mybir.dt.bfloat16
AFT = mybir.ActivationFunctionType
ALU = mybir.AluOpType

B_FULL = 8192
UNITS_FULL = 4096
N_IN = 4096

K_EFF = 256                     # truncated contraction dim (see module docstring)
SHARD_B = 2                     # batch split
SHARD_U = 4                     # units split
B_LOC = B_FULL // SHARD_B       # 4096
U_LOC = UNITS_FULL // SHARD_U   # 1024

P = 128
K_TILES = [(0, 128), (128, 128)]   # (offset, size), sums to K_EFF
NB = B_LOC // 512               # 8 moving-dim chunks of 512
NU = U_LOC // P                 # 8 unit partition tiles

_NC_CACHE = None


def _build_nc():
    nc = bass.Bass()
    xt_d = nc.declare_dram_parameter("xt", [2 * K_EFF, B_LOC], BF16, isOutput=False)
    th_d = nc.declare_dram_parameter("theta", [U_LOC, K_EFF + 2], F32, isOutput=False)
    lamr_d = nc.declare_dram_parameter("lamr", [2, U_LOC + 512], BF16, isOutput=False)
    out_d = nc.declare_dram_parameter("out", [U_LOC, B_LOC], F32, isOutput=True)

    # This walrus build fits at most ONE fused semaphore wait on compute
    # instructions (optimize_sems is disabled upstream), so the whole kernel
    # is arranged so every compute op's dependencies collapse onto a single
    # semaphore:
    #  * everything the PE consumes (v, xt, lamr) is produced by DVE copies,
    #    so matmuls only ever wait on the DVE semaphore (or a PSUM-bank WAR
    #    that is also DVE);
    #  * the PSUM->SBUF eviction is one fused DVE op (max 0, * r) whose PE
    #    and self dependencies are pre-observed via two 1-element "absorber"
    #    DVE copies, leaving only the output-slot DMA WAR on it;
    #  * DMAs are issued by gpsimd (SWDGE), which fits two waits.
    with ExitStack() as ctx:
        tc = ctx.enter_context(tile.TileContext(nc))
        const = ctx.enter_context(tc.tile_pool(name="const", bufs=1))
        thpool = ctx.enter_context(tc.tile_pool(name="th", bufs=1))
        vpool = ctx.enter_context(tc.tile_pool(name="v", bufs=1))
        xpool = ctx.enter_context(tc.tile_pool(name="x", bufs=1))
        xstage = ctx.enter_context(tc.tile_pool(name="xstage", bufs=3))
        work = ctx.enter_context(tc.tile_pool(name="work", bufs=8))
        psum = ctx.enter_context(tc.tile_pool(name="ps", bufs=5, space="PSUM"))
        psync = ctx.enter_context(tc.tile_pool(name="psync", bufs=1, space="PSUM"))
        psum_tr = ctx.enter_context(tc.tile_pool(name="pstr", bufs=2, space="PSUM"))
        opool = ctx.enter_context(tc.tile_pool(name="o", bufs=1))
        scr = ctx.enter_context(tc.tile_pool(name="scr", bufs=1))

        ident0 = const.tile([P, P], F32, tag="ident0")
        make_identity(nc, ident0[:])
        # DVE-bounced identity: transposes are self-loading fp32 matmuls with
        # a single LDWEIGHTS sync slot, so both their deps must be DVE.
        ident = const.tile([P, P], F32, tag="ident")
        nc.vector.tensor_copy(ident[:], ident0[:])
        halfpi = const.tile([P, 1], F32, tag="halfpi")
        nc.vector.memset(halfpi[:], float(np.pi / 2))

        # bf16 hi+lo split operands: z = xh@vh + xh@vl + xl@vh reproduces the
        # fp32 product to ~2^-18 relative (the xl@vl term is negligible).
        vh_sb, vl_sb, xh_sb, xl_sb = [], [], [], []
        for k, (ko, kp) in enumerate(K_TILES):
            vh_sb.append(vpool.tile([kp, U_LOC], BF16, tag=f"vh{k}", name=f"vh{k}"))
            vl_sb.append(vpool.tile([kp, U_LOC], BF16, tag=f"vl{k}", name=f"vl{k}"))
            xh_sb.append(xpool.tile([kp, B_LOC], BF16, tag=f"xh{k}", name=f"xh{k}"))
            xl_sb.append(xpool.tile([kp, B_LOC], BF16, tag=f"xl{k}", name=f"xl{k}"))
        th_tiles = [thpool.tile([P, K_EFF + 2], F32, tag=f"th{u}", name=f"th{u}")
                    for u in range(NU)]

        # xt and lamr arrive from DRAM already rounded to f32r (tf32) by the
        # host, so they DMA straight into f32r tiles with no data-dependency
        # wait (their loads then carry only the queue-slot wait).
        # ALL input loads go through gpsimd (SWDGE procs) so that the eight
        # big output DMAs can be the first-and-only users of the HWDGE procs
        # (a DMA instruction fits one sync wait: first-per-proc DMAs can
        # carry their data wait, later ones cannot).
        CHUNK = 2048
        for k, (ko, kp) in enumerate(K_TILES):
            for c in range(B_LOC // CHUNK):
                cs = c * CHUNK
                nc.gpsimd.dma_start(xh_sb[k][:, cs:cs + CHUNK],
                                    xt_d[ko:ko + kp, cs:cs + CHUNK])
                nc.gpsimd.dma_start(xl_sb[k][:, cs:cs + CHUNK],
                                    xt_d[K_EFF + ko:K_EFF + ko + kp, cs:cs + CHUNK])
        for u in range(NU):
            nc.gpsimd.dma_start(th_tiles[u][:], th_d[u * P:(u + 1) * P, :])
        lamr_r = thpool.tile([2, U_LOC + 512], BF16, tag="lamr_r", name="lamr_r")
        nc.gpsimd.dma_start(lamr_r[:], lamr_d[:])
        # r columns, DVE-bounced so the fused eviction op's scalar is DVE.
        rd_tiles = []
        for u in range(NU):
            rd = thpool.tile([P, 1], F32, tag=f"rd{u}", name=f"rd{u}")
            nc.vector.tensor_copy(rd[:], th_tiles[u][:, K_EFF + 1:K_EFF + 2])
            rd_tiles.append(rd)

        # ---- Phase A: build v (K x U layout, f32r) from angles ----------
        for u in range(NU):
            ang = th_tiles[u][:, 0:K_EFF]
            sin_t = work.tile([P, K_EFF], F32, tag="sin")
            nc.scalar.activation(sin_t[:], ang, AFT.Sin)
            cos_t = work.tile([P, K_EFF], F32, tag="cos")
            nc.scalar.activation(cos_t[:], ang, AFT.Sin, bias=halfpi[:])
            # scp[:, i] = cumprod(sin)[:, i-1], scp[:, 0] = 1
            scp = work.tile([P, K_EFF], F32, tag="scp")
            nc.vector.memset(scp[:, 0:1], 1.0)
            nc.vector.tensor_tensor_scan(
                scp[:, 1:K_EFF], sin_t[:, 0:K_EFF - 1], sin_t[:, 0:K_EFF - 1],
                1.0, ALU.mult, ALU.bypass,
            )
            # cos bounced through DVE, then vT = cos * scp in-place (all-DVE)
            cosd = work.tile([P, K_EFF], F32, tag="cosd")
            nc.vector.tensor_copy(cosd[:], cos_t[:])
            nc.vector.scalar_tensor_tensor(
                cosd[:], cosd[:], 0.0, scp[:], ALU.bypass, ALU.mult)
            for k, (ko, kp) in enumerate(K_TILES):
                pst = psum_tr.tile([P, 128], F32, tag="pstr")
                nc.tensor.transpose(pst[:kp, :P], cosd[:, ko:ko + kp], ident[:])
                vf = work.tile([P, 128], F32, tag="vf",
                               bufs=len(K_TILES) * NU, name=f"vf{u}_{k}")
                nc.vector.tensor_copy(vf[:kp, :], pst[:kp, :P])
                usl = slice(u * P, (u + 1) * P)
                nc.vector.tensor_copy(vh_sb[k][:, usl], vf[:kp, :])
                nc.vector.scalar_tensor_tensor(
                    vl_sb[k][:, usl], vf[:kp, :], 0.0, vh_sb[k][:, usl],
                    ALU.bypass, ALU.subtract)

        # ---- Phase B: psum = lam + v^T x ; zo = r * max(psum, 0) --------
        # Per tile, after the accumulation matmuls, a zero-contribution 1x1
        # matmul (0 * scrx[n]) closes the group; the "dpe" absorber then
        # OVERWRITES scrx[n], giving DVE a pure WAR wait on [PE >= stop] with
        # no PSUM read (PSUM reads from two DVE ops would get serialized by
        # the bank tracker with an extra sync edge).  "dself" refreshes DVE's
        # observed self-clock.  The fused eviction then emits only the
        # output-slot DMA WAR: exactly one hw sync wait.
        NTILE = NU * NB
        zsrc = const.tile([1, 1], BF16, tag="zsrc")
        nc.vector.memset(zsrc[:], 0.0)
        scrxf = const.tile([1, NTILE], BF16, tag="scrxf")
        nc.vector.memset(scrxf[:], 0.0)
        scry = const.tile([1, NTILE], F32, tag="scry")
        nc.vector.memset(scry[:], 0.0)

        # One marker matmul reading the last-written v_sb region: after it,
        # PE's observed DVE clock covers every v_sb copy, so phase-B k-MMs
        # only ever emit the xt-load DMA wait (one semaphore each).
        first_mm = None
        ptv = psync.tile([1, 16], F32, tag="psync")
        vsync = nc.tensor.matmul(
            ptv[0:1, 0:1], zsrc[:],
            vl_sb[len(K_TILES) - 1][0:1, U_LOC - 1:U_LOC],
            start=True, stop=True,
        )

        prev_zo = ident
        prev_reg = ident[0:1, 0:1]
        n = 0
        for u in range(NU):
            r_col = rd_tiles[u][:]
            zrow = opool.tile([P, B_LOC], F32, tag="zrow", bufs=3)
            for nb in range(NB):
                pt = psum.tile([P, 512], F32, tag="ps")
                bias_mm = nc.tensor.matmul(
                    pt[:],
                    lamr_r[0:2, u * P:(u + 1) * P],
                    lamr_r[0:2, U_LOC:U_LOC + 512],
                    start=True, stop=False,
                )
                if first_mm is None:
                    first_mm = bias_mm.ins
                usl = slice(u * P, (u + 1) * P)
                bsl = slice(nb * 512, (nb + 1) * 512)
                lastmm = None
                for k, (ko, kp) in enumerate(K_TILES):
                    nc.tensor.matmul(pt[:], vh_sb[k][:, usl], xh_sb[k][:, bsl],
                                     start=False, stop=False)
                    nc.tensor.matmul(pt[:], vl_sb[k][:, usl], xh_sb[k][:, bsl],
                                     start=False, stop=False)
                    lastmm = nc.tensor.matmul(
                        pt[:], vh_sb[k][:, usl], xl_sb[k][:, bsl],
                        start=False, stop=(k == len(K_TILES) - 1),
                    )
                # Zero 1x1 matmul into a scratch PSUM tile: PE is in-order, so
                # its completion implies the whole group is done; the dpe
                # absorber then takes a pure [PE >= here] WAR by overwriting
                # scrx[n], without any DVE read of PSUM.
                pt5 = psum_tr.tile([P, 128], F32, tag="pstr")
                mm5 = nc.tensor.matmul(
                    pt5[0:1, 0:1], zsrc[:], scrxf[0:1, n:n + 1],
                    start=True, stop=True,
                )
                add_dep_helper(mm5.ins, lastmm.ins, sync=False,
                               reason="order PE-marker after accumulation")
                dpe = nc.vector.tensor_copy(scrxf[0:1, n:n + 1], zsrc[:])
                dself = nc.vector.tensor_copy(scry[0:1, n:n + 1], prev_reg)
                zslice = zrow[:, nb * 512:(nb + 1) * 512]
                fused = nc.vector.tensor_scalar(
                    zslice, pt[:], 0.0, r_col, ALU.max, ALU.mult)
                if n == 0 and first_mm is not None:
                    add_dep_helper(first_mm, vsync.ins, sync=False,
                                   reason="order v_sb marker before phase B")
                add_dep_helper(fused.ins, dpe.ins, sync=False,
                               reason="absorb PE wait before eviction")
                add_dep_helper(fused.ins, dself.ins, sync=False,
                               reason="absorb DVE self wait before eviction")
                prev_reg = zrow[0:1, nb * 512:nb * 512 + 1]
                n += 1
            nc.sync.dma_start(out_d[u * P:(u + 1) * P, :], zrow[:])
    return nc


def _split_excess_waits(nc, max_waits=1):
    """walrus refuses instructions whose descriptor carries more than one
    fused semaphore wait.  Hoist all but the last wait of any such
    instruction into standalone EventSemaphore instructions inserted just
    before it on the same engine queue — semantically identical (the engine
    blocks on the standalone waits first)."""
    ctr = 0
    for f in nc.m.functions:
        for bb in f.blocks:
            insts = bb.instructions
            i = 0
            while i < len(insts):
                ins = insts[i]
                si = ins.sync_info
                if si is not None and len(si.on_wait) > max_waits:
                    keep = si.on_wait[-max_waits:]
                    hoist = si.on_wait[:-max_waits]
                    pos = i
                    for w in hoist:
                        ev = mybir.InstEventSemaphore(
                            name=f"evsplit-{ctr}", ins=[], outs=[])
                        ctr += 1
                        ev.engine = ins.engine
                        ev.sync_info = mybir.SyncInfo(on_wait=[w], on_update=[])
                        nc.register_instruction(ev, overwrite=True)
                        insts.insert(pos, ev)
                        pos += 1
                        i += 1
                    ins.sync_info = mybir.SyncInfo(
                        on_wait=list(keep), on_update=list(si.on_update))
                i += 1
    return nc


def get_nc():
    global _NC_CACHE
    if _NC_CACHE is None:
        _NC_CACHE = _split_excess_waits(_build_nc())
    return _NC_CACHE


import ml_dtypes

BF16_NP = ml_dtypes.bfloat16


def bf16_split(a: np.ndarray):
    """Split fp32 into bf16 hi + lo with hi + lo ~= a to ~2^-17 relative."""
    a = np.ascontiguousarray(a, dtype=np.float32)
    hi = a.astype(BF16_NP)
    lo = (a - hi.astype(np.float32)).astype(BF16_NP)
    return hi, lo


def _check_truncation(theta_lambda: np.ndarray):
    s = np.sin(theta_lambda[:K_EFF].astype(np.float32), dtype=np.float32)
    cp = np.cumprod(s, axis=0, dtype=np.float32)
    if np.abs(cp[K_EFF - 16:]).max() != 0.0:
        raise ValueError(
            "cumprod(sin(angles)) did not underflow to zero before row "
            f"{K_EFF - 16}: the K_EFF={K_EFF} truncation is unsafe for "
            "these inputs")


def make_in_maps(x: np.ndarray, theta_lambda: np.ndarray):
    x = np.ascontiguousarray(x, dtype=np.float32)
    theta_lambda = np.ascontiguousarray(theta_lambda, dtype=np.float32)
    _check_truncation(theta_lambda)
    in_maps = []
    xt_halves = []
    for b in range(SHARD_B):
        hi, lo = bf16_split(x[b * B_LOC:(b + 1) * B_LOC, :K_EFF].T)
        xt = np.empty((2 * K_EFF, B_LOC), dtype=BF16_NP)
        xt[:K_EFF] = hi
        xt[K_EFF:] = lo
        xt_halves.append(xt)
    for core in range(SHARD_B * SHARD_U):
        b, g = divmod(core, SHARD_U)
        us = g * U_LOC
        ue = us + U_LOC
        theta_t = np.empty((U_LOC, K_EFF + 2), dtype=np.float32)
        theta_t[:, :K_EFF] = theta_lambda[:K_EFF, us:ue].T
        theta_t[:, K_EFF] = theta_lambda[N_IN - 1, us:ue]       # lambda row
        theta_t[:, K_EFF + 1] = theta_lambda[N_IN, us:ue]       # radius row
        lamh, laml = bf16_split(theta_lambda[N_IN - 1, us:ue])
        lamr = np.empty((2, U_LOC + 512), dtype=BF16_NP)
        lamr[0, :U_LOC] = lamh
        lamr[1, :U_LOC] = laml
        lamr[0, U_LOC:] = 1.0
        lamr[1, U_LOC:] = 1.0
        in_maps.append({"xt": xt_halves[b], "theta": theta_t, "lamr": lamr})
    return in_maps


def assemble(results) -> np.ndarray:
    out = np.empty((B_FULL, UNITS_FULL), dtype=np.float32)
    for core, res in enumerate(results):
        b, g = divmod(core, SHARD_U)
        out[b * B_LOC:(b + 1) * B_LOC, g * U_LOC:(g + 1) * U_LOC] = res["out"].T
    return out


def kernel(x: np.ndarray, theta_lambda: np.ndarray) -> np.ndarray:
    nc = get_nc()
    in_maps = make_in_maps(x, theta_lambda)
    res = run_bass_kernel_spmd(nc, in_maps, list(range(SHARD_B * SHARD_U)))
    return assemble(res.results)


if __name__ == "__main__":
    rng = np.random.default_rng(0)
    x = rng.standard_normal((B_FULL, N_IN), dtype=np.float32)
    tl = rng.standard_normal((N_IN + 1, UNITS_FULL), dtype=np.float32)
    out = kernel(x, tl)
    print("out", out.shape, out.dtype, float(np.abs(out).max()))



# revision 5
# speedup vs baseline: 4031.1672x; 4031.1672x over previous
"""Trainium2 Bass kernel for nn_DenseReparam.

Reference computation (fp32):
    angles = theta_lambda[:-2]            # [4095, 4096]
    lam    = theta_lambda[-2]             # [4096]
    r      = theta_lambda[-1]             # [4096]
    s, c   = sin(angles), cos(angles)
    cp     = cumprod(s, axis=0)
    v      = [c[0]; c[1:]*cp[:-1]; cp[-1]]   # [4096, 4096]
    z      = x @ v + lam                     # [8192, 4096]
    out    = r * relu(z)

Key numerical fact exploited: cp decays like exp(-0.75*k) (angles are standard
normal), so in fp32 cp underflows to exactly 0 by row ~231 for every column.
All v rows >= 232 are exact zeros and contribute nothing to x @ v, so the
contraction dim truncates from 4096 to K_EFF = 254 (verified at runtime).

Precision budget (gate is rel_err < 2e-2): single bf16 matmul pass costs
~2.3e-3, the ACT Sin LUT ~3.5e-3, bf16 output rounding ~2e-3 -> ~5e-3 total.
That lets the kernel run ONE bf16 matmul per output tile instead of the 3
hi/lo-split passes of the earlier version, with a bf16 (not fp32) result
written back to HBM.

The lam bias rides the matmul for free: the two stationary chunks are
[v rows 0..125; lam_hi; lam_lo] and [v rows 126..253] (254 + 2 = 256 = 2x128),
with the moving operand carrying two rows of ones in the matching slots.
No separate bias matmul, no bias pass in the epilogue; the eviction is a
single fused DVE op  out = r * max(psum, 0)  per [128 x 512] tile.

Sharding (8 cores): batch split 2 x units split 4.  Each core computes
zT_local [1024 units, 4096 batch] in bf16; host reassembles out[b, g] =
zT_local^T (fp32 upcast).
"""

import sys

import numpy as np

for _p in ("/root/.axon_site", "/root/.axon_site/_ro/trn_rl_repo",
           "/root/.axon_site/_ro/pypackages", "/opt/trn_rl_repo"):
    if _p not in sys.path:
        sys.path.append(_p)

from contextlib import ExitStack

from concourse import bass, mybir, tile
from concourse.bass_utils import run_bass_kernel_spmd
from concourse.masks import make_identity

F32 = mybir.dt.float32
BF16 = mybir.dt.bfloat16
AFT = mybir.ActivationFunctionType
ALU = mybir.AluOpType

B_FULL = 8192
UNITS_FULL = 4096
N_IN = 4096

K_EFF = 254                     # truncated contraction dim (see module docstring)
K0 = 126                        # v rows in stationary chunk 0 (+2 lam rows = 128)
SHARD_B = 2                     # batch split
SHARD_U = 4                     # units split
B_LOC = B_FULL // SHARD_B       # 4096
U_LOC = UNITS_FULL // SHARD_U   # 1024

P = 128
NB = B_LOC // 512               # 8 moving-dim chunks of 512
NU = U_LOC // P                 # 8 unit partition tiles

_NC_CACHE = {}


def _build_nc(repeat=1):
    nc = bass.Bass()
    xt_d = nc.declare_dram_parameter("xt", [2 * P, B_LOC], BF16, isOutput=False)
    th_d = nc.declare_dram_parameter("theta", [U_LOC, K_EFF + 2], F32, isOutput=False)
    out_d = nc.declare_dram_parameter("out", [U_LOC, B_LOC], BF16, isOutput=True)

    with ExitStack() as ctx:
        tc = ctx.enter_context(tile.TileContext(nc))
        const = ctx.enter_context(tc.tile_pool(name="const", bufs=1))
        thpool = ctx.enter_context(tc.tile_pool(name="th", bufs=1))
        vpool = ctx.enter_context(tc.tile_pool(name="v", bufs=1))
        xpool = ctx.enter_context(tc.tile_pool(name="x", bufs=1))
        work = ctx.enter_context(tc.tile_pool(name="work", bufs=3))
        psum = ctx.enter_context(tc.tile_pool(name="ps", bufs=6, space="PSUM"))
        psum_tr = ctx.enter_context(tc.tile_pool(name="pstr", bufs=2, space="PSUM"))
        opool = ctx.enter_context(tc.tile_pool(name="o", bufs=3))

        ident0 = const.tile([P, P], F32, tag="ident0")
        make_identity(nc, ident0[:])
        # DVE-bounced identity keeps the transpose matmuls' deps on one engine.
        ident = const.tile([P, P], F32, tag="ident")
        nc.vector.tensor_copy(ident[:], ident0[:])
        halfpi = const.tile([P, 1], F32, tag="halfpi")
        nc.vector.memset(halfpi[:], float(np.pi / 2))

        for _ in range(repeat):
            # ---- input loads -------------------------------------------
            x_sb = []
            for k in range(2):
                xk = xpool.tile([P, B_LOC], BF16, tag=f"x{k}")
                for c in range(2):
                    cs = c * (B_LOC // 2)
                    nc.gpsimd.dma_start(xk[:, cs:cs + B_LOC // 2],
                                        xt_d[k * P:(k + 1) * P, cs:cs + B_LOC // 2])
                x_sb.append(xk)
            th_tiles = []
            for u in range(NU):
                th = thpool.tile([P, K_EFF + 2], F32, tag=f"th{u}")
                nc.gpsimd.dma_start(th[:], th_d[u * P:(u + 1) * P, :])
                th_tiles.append(th)

            vh0 = vpool.tile([P, U_LOC], BF16, tag="vh0")
            vh1 = vpool.tile([P, U_LOC], BF16, tag="vh1")
            rd_tiles = []

            for u in range(NU):
                usl = slice(u * P, (u + 1) * P)
                th = th_tiles[u]
                # ---- phase A(u): build v columns for this u-tile -------
                ang = th[:, 0:K_EFF]
                sin_t = work.tile([P, K_EFF], F32, tag="sin")
                nc.scalar.activation(sin_t[:], ang, AFT.Sin)
                cos_t = work.tile([P, K_EFF], F32, tag="cos")
                nc.scalar.activation(cos_t[:], ang, AFT.Sin, bias=halfpi[:])
                # scp[:, i] = cumprod(sin)[:, i-1], scp[:, 0] = 1
                scp = work.tile([P, K_EFF], F32, tag="scp")
                nc.vector.memset(scp[:, 0:1], 1.0)
                nc.vector.tensor_tensor_scan(
                    scp[:, 1:K_EFF], sin_t[:, 0:K_EFF - 1], sin_t[:, 0:K_EFF - 1],
                    1.0, ALU.mult, ALU.bypass,
                )
                # vT in units-major layout, packed for the two transposes:
                # vTa cols = [v[0:126], lam_hi, lam_lo], vTb cols = v[126:254].
                # lam splits into bf16 hi+lo columns BEFORE the transpose so no
                # op needs a non-32-aligned partition base afterwards.
                vta = work.tile([P, P], F32, tag="vta")
                nc.vector.scalar_tensor_tensor(
                    vta[:, 0:K0], cos_t[:, 0:K0], 0.0, scp[:, 0:K0],
                    ALU.bypass, ALU.mult)
                lamh = work.tile([P, 1], BF16, tag="lamh")
                nc.vector.tensor_copy(lamh[:], th[:, K_EFF:K_EFF + 1])
                nc.vector.tensor_copy(vta[:, K0:K0 + 1], lamh[:])
                nc.vector.scalar_tensor_tensor(
                    vta[:, K0 + 1:K0 + 2], th[:, K_EFF:K_EFF + 1], 0.0,
                    lamh[:], ALU.bypass, ALU.subtract)
                vtb = work.tile([P, P], F32, tag="vtb")
                nc.vector.scalar_tensor_tensor(
                    vtb[:], cos_t[:, K0:K_EFF], 0.0, scp[:, K0:K_EFF],
                    ALU.bypass, ALU.mult)

                pst_a = psum_tr.tile([P, P], F32, tag="pstr")
                nc.tensor.transpose(pst_a[:], vta[:], ident[:])
                pst_b = psum_tr.tile([P, P], F32, tag="pstr")
                nc.tensor.transpose(pst_b[:], vtb[:], ident[:])

                nc.vector.tensor_copy(vh0[:, usl], pst_a[:])
                nc.vector.tensor_copy(vh1[:, usl], pst_b[:])

                rd = thpool.tile([P, 1], F32, tag=f"rd{u}")
                nc.vector.tensor_copy(rd[:], th[:, K_EFF + 1:K_EFF + 2])
                rd_tiles.append(rd)

                # ---- phase B(u): z = v^T x (+lam), out = r*max(z,0) ----
                zrow = opool.tile([P, B_LOC], BF16, tag="zrow")
                for g in range(2):
                    nbs = range(g * 4, g * 4 + 4)
                    pts = [psum.tile([P, 512], F32, tag="pb", name=f"pb{u}_{g}_{j}")
                           for j in range(4)]
                    for k in range(2):
                        vk = vh0 if k == 0 else vh1
                        for j, nb in enumerate(nbs):
                            bsl = slice(nb * 512, (nb + 1) * 512)
                            nc.tensor.matmul(
                                pts[j][:], vk[:, usl], x_sb[k][:, bsl],
                                start=(k == 0), stop=(k == 1))
                    for j, nb in enumerate(nbs):
                        bsl = slice(nb * 512, (nb + 1) * 512)
                        nc.vector.tensor_scalar(
                            zrow[:, bsl], pts[j][:], 0.0, rd_tiles[u][:],
                            ALU.max, ALU.mult)
                nc.sync.dma_start(out_d[u * P:(u + 1) * P, :], zrow[:])
    return nc


def _split_excess_waits(nc, max_waits=1):
    """walrus refuses instructions whose descriptor carries more than one
    fused semaphore wait.  Hoist all but the last wait of any such
    instruction into standalone EventSemaphore instructions inserted just
    before it on the same engine queue — semantically identical (the engine
    blocks on the standalone waits first)."""
    ctr = 0
    for f in nc.m.functions:
        for bb in f.blocks:
            insts = bb.instructions
            i = 0
            while i < len(insts):
                ins = insts[i]
                si = ins.sync_info
                if si is not None and len(si.on_wait) > max_waits:
                    keep = si.on_wait[-max_waits:]
                    hoist = si.on_wait[:-max_waits]
                    pos = i
                    for w in hoist:
                        ev = mybir.InstEventSemaphore(
                            name=f"evsplit-{ctr}", ins=[], outs=[])
                        ctr += 1
                        ev.engine = ins.engine
                        ev.sync_info = mybir.SyncInfo(on_wait=[w], on_update=[])
                        nc.register_instruction(ev, overwrite=True)
                        insts.insert(pos, ev)
                        pos += 1
                        i += 1
                    ins.sync_info = mybir.SyncInfo(
                        on_wait=list(keep), on_update=list(si.on_update))
                i += 1
    return nc


def get_nc(repeat=1):
    if repeat not in _NC_CACHE:
        _NC_CACHE[repeat] = _split_excess_waits(_build_nc(repeat))
    return _NC_CACHE[repeat]


import ml_dtypes

BF16_NP = ml_dtypes.bfloat16


def _check_truncation(theta_lambda: np.ndarray):
    s = np.sin(theta_lambda[:K_EFF].astype(np.float32), dtype=np.float32)
    cp = np.cumprod(s, axis=0, dtype=np.float32)
    if np.abs(cp[K_EFF - 16:]).max() != 0.0:
        raise ValueError(
            "cumprod(sin(angles)) did not underflow to zero before row "
            f"{K_EFF - 16}: the K_EFF={K_EFF} truncation is unsafe for "
            "these inputs")


def make_in_maps(x: np.ndarray, theta_lambda: np.ndarray):
    x = np.ascontiguousarray(x, dtype=np.float32)
    theta_lambda = np.ascontiguousarray(theta_lambda, dtype=np.float32)
    _check_truncation(theta_lambda)
    in_maps = []
    xt_halves = []
    for b in range(SHARD_B):
        xb = x[b * B_LOC:(b + 1) * B_LOC, :K_EFF].T  # [254, B_LOC]
        xt = np.empty((2 * P, B_LOC), dtype=BF16_NP)
        xt[0:K0] = xb[0:K0]
        xt[K0:P] = 1.0                     # pairs with the lam_hi/lam_lo rows
        xt[P:2 * P] = xb[K0:K_EFF]
        xt_halves.append(xt)
    for core in range(SHARD_B * SHARD_U):
        b, g = divmod(core, SHARD_U)
        us = g * U_LOC
        ue = us + U_LOC
        theta_t = np.empty((U_LOC, K_EFF + 2), dtype=np.float32)
        theta_t[:, :K_EFF] = theta_lambda[:K_EFF, us:ue].T
        theta_t[:, K_EFF] = theta_lambda[N_IN - 1, us:ue]       # lambda row
        theta_t[:, K_EFF + 1] = theta_lambda[N_IN, us:ue]       # radius row
        in_maps.append({"xt": xt_halves[b], "theta": theta_t})
    return in_maps


def assemble(results) -> np.ndarray:
    out = np.empty((B_FULL, UNITS_FULL), dtype=np.float32)
    for core, res in enumerate(results):
        b, g = divmod(core, SHARD_U)
        out[b * B_LOC:(b + 1) * B_LOC, g * U_LOC:(g + 1) * U_LOC] = \
            res["out"].T.astype(np.float32)
    return out


def kernel(x: np.ndarray, theta_lambda: np.ndarray) -> np.ndarray:
    nc = get_nc()
    in_maps = make_in_maps(x, theta_lambda)
    res = run_bass_kernel_spmd(nc, in_maps, list(range(SHARD_B * SHARD_U)))
    return assemble(res.results)


if __name__ == "__main__":
    rng = np.random.default_rng(0)
    x = rng.standard_normal((B_FULL, N_IN), dtype=np.float32)
    tl = rng.standard_normal((N_IN + 1, UNITS_FULL), dtype=np.float32)
    out = kernel(x, tl)
    print("out", out.shape, out.dtype, float(np.abs(out).max()))
